# revision 4
# baseline (speedup 1.0000x reference)
"""Trainium2 Bass kernel for nn_DynamicFilter (dynamic per-image 3x3 grouped filter).

Math (per batch n, channel c, group g = c//4):
    pooled[n,c] = mean_hw x[n,c]
    f = pooled @ W2.T + b2          (conv1x1 + folded BN)
    filt[n,g,k] = tanh(f)           (k = 9 taps, 3x3, reflect pad)
    out = A_c * conv3x3_{filt[g]}(x) + s_c * x + Bc_c * pooled[n,c]
      A = lamb_l*(inside_all+1), s = lamb_h+1, Bc = -lamb_l*inside_all

Sharding: 8 cores = (n in 0..3) x (channel half in 0..1), 16 channels/core.
The pooled mean needs all 32 channels per n -> tiny pair AllGather (16 floats).

Device mapping per core:
  - x rows (H) on SBUF partitions, one window-tile set per channel with
    1-row overlaps and reflection rows/cols materialized at load.
  - 3x3 conv = 3 matmuls per channel accumulating in PSUM: each lhsT is a
    tridiagonal [in_row x out_row] matrix carrying the 3 vertical taps for
    one horizontal shift dx; rhs free-dim offset provides dx.
  - residual s*x folded into the center tridiagonal diagonal as sigma=s/A;
    per-channel scale A and bias Bc*pooled applied by the ScalarE copy that
    evacuates PSUM.
  - pooling: per-window ones-vector matmuls -> column sums in PSUM ->
    free-dim reduce -> [16,1] partial -> pair AllGather -> [32,1].
"""

import numpy as np

import concourse.bass as bass
import concourse.mybir as mybir
import concourse.tile as tile
from concourse import bacc, bass_utils

F32 = mybir.dt.float32

N_B, C, H, W = 4, 32, 256, 256
CPC = 16   # channels per core
NCORES = 8
EPS = 1e-5

# smallrow / broadcast-table column layout
FCOL = 0     # filt (own 4 groups x 9 taps) : 36
ACOL = 36    # A_eff per own channel        : 16
SIGCOL = 52  # sigma = s/A_eff              : 16
BCCOL = 68   # Bc = -lamb_l*inside_all      : 16
SCOL = 84    # s = lamb_h+1 (unused on dev) : 16
POOLCOL = 100  # own pooled sums            : 16
SRW = 116

# window geometry: (main_src_row0, main_nrows, main_dst_part0,
#                   reflect_src_row, reflect_dst_part,
#                   pool_part0, pool_part1, out_row0, out_nrows, in_parts)
WINDOWS = [
    (0, 87, 1, 1, 0, 1, 87, 0, 86, 88),     # rows -1..86 (part0 = reflect row 1)
    (85, 87, 0, None, None, 1, 86, 86, 85, 87),
    (170, 86, 0, 254, 86, 1, 86, 171, 85, 87),  # part86 = reflect row 254
]
WBLK = 264  # column stride of one window block inside a channel tile
# pooling selector block inside the shifts constant: M[p, 384 + 15 + 16w] is
# the 0/1 row-valid mask of window w; slicing 16 cols starting at
# 384 + 15 + 16w - ch puts that mask at relative column ch, zeros elsewhere.
SELBASE = 384
SHIFTS_W = 448


def _build_nc():
    nc = bacc.Bacc()
    xs = nc.declare_dram_parameter("xs", [CPC, H, W], F32, isOutput=False)
    w2t = nc.declare_dram_parameter("w2t", [C, 36], F32, isOutput=False)
    params = nc.declare_dram_parameter("params", [1, 100], F32, isOutput=False)
    shifts = nc.declare_dram_parameter("shifts", [128, SHIFTS_W], F32, isOutput=False)
    out_d = nc.declare_dram_parameter("out", [CPC, H, W], F32, isOutput=True)

    cc_in = nc.dram_tensor("cc_in", [CPC, 1], F32)
    cc_out = nc.dram_tensor("cc_out", [C, 1], F32)

    with tile.TileContext(nc) as tc:
        with (
            tc.tile_pool(name="wbig", bufs=1) as wpool,
            tc.tile_pool(name="consts", bufs=1) as cpool,
            tc.tile_pool(name="small", bufs=1) as spool,
            tc.tile_pool(name="tri", bufs=1) as tripool,
            tc.tile_pool(name="tritmp", bufs=2) as tmppool,
            tc.tile_pool(name="outs", bufs=3) as opool,
            tc.tile_pool(name="ps_small", bufs=1, space="PSUM") as pspool,
            tc.tile_pool(name="ps_a", bufs=2, space="PSUM") as psa_pool,
            tc.tile_pool(name="ps_b", bufs=2, space="PSUM") as psb_pool,
        ):
            # constants
            shifts_t = cpool.tile([128, SHIFTS_W], F32, tag="shifts")
            nc.sync.dma_start(shifts_t[:, :], shifts[:, :])
            w2t_t = cpool.tile([C, 36], F32, tag="w2t")
            nc.sync.dma_start(w2t_t[:, :], w2t[:, :])
            ptile = cpool.tile([1, 100], F32, tag="ptile")
            nc.sync.dma_start(ptile[:, :], params[:, :])
            onescol = cpool.tile([128, 1], F32, tag="onescol")
            nc.vector.memset(onescol[:, :], 1.0)
            onesrow = cpool.tile([1, 128], F32, tag="onesrow")
            nc.vector.memset(onesrow[:, :], 1.0)
            smallrow = cpool.tile([1, SRW], F32, tag="smallrow")
            nc.sync.dma_start(smallrow[0:1, ACOL:SCOL + 16], ptile[0:1, 36:100])

            # per-channel window tiles + load + pooling column sums
            wch = []
            poolp = pspool.tile([CPC, W], F32, tag="poolp")
            for ch in range(CPC):
                t = wpool.tile([128, 3 * WBLK], F32, tag=f"wch{ch}")
                wch.append(t)
                for wi, (r0, nr, p0, rr, rp, pv0, pv1, _, _, _) in enumerate(WINDOWS):
                    base = wi * WBLK
                    nc.sync.dma_start(
                        t[p0:p0 + nr, base + 1:base + 257],
                        xs[ch, r0:r0 + nr, :],
                    )
                    if rr is not None:
                        nc.sync.dma_start(
                            t[rp:rp + 1, base + 1:base + 257],
                            xs[ch, rr:rr + 1, :],
                        )
                # reflect columns: col0 <- col2, col257 <- col255 (all 3 blocks)
                t3 = t[0:88, :].rearrange("p (w c) -> p w c", c=WBLK)
                nc.vector.tensor_copy(t3[:, 0:3, 0:1], t3[:, 0:3, 2:3])
                nc.vector.tensor_copy(t3[:, 0:3, 257:258], t3[:, 0:3, 255:256])
                # pooling: per-window masked column sums accumulated into
                # poolp partition ch (selector column zero elsewhere)
                for wi, (_, _, _, _, _, _, _, _, _, nparts) in enumerate(WINDOWS):
                    base = wi * WBLK
                    sel0 = SELBASE + 15 + 16 * wi - ch
                    nc.tensor.matmul(
                        poolp[0:CPC, :],
                        shifts_t[0:nparts, sel0:sel0 + 16],
                        wch[ch][0:nparts, base + 1:base + 257],
                        start=(ch == 0 and wi == 0),
                        stop=(ch == CPC - 1 and wi == 2),
                    )

            # finish pooling: CS -> reduce -> AllGather
            cs = spool.tile([CPC, W], F32, tag="cs")
            nc.scalar.activation(cs[:, :], poolp[:, :], mybir.ActivationFunctionType.Copy)
            pool_own = spool.tile([CPC, 1], F32, tag="pool_own")
            nc.vector.tensor_reduce(
                pool_own[:, :], cs[:, :], axis=mybir.AxisListType.X, op=mybir.AluOpType.add
            )
            nc.sync.dma_start(cc_in[:, :], pool_own[:, :])
            nc.gpsimd.collective_compute(
                "AllGather",
                mybir.AluOpType.bypass,
                replica_groups=[[0, 1], [2, 3], [4, 5], [6, 7]],
                ins=[cc_in.ap().opt()],
                outs=[cc_out.ap().opt()],
            )
            pooled_col = spool.tile([C, 1], F32, tag="pooled_col")
            nc.sync.dma_start(pooled_col[:, :], cc_out[:, :])

            # own pooled sums as a row (overlaps the collective)
            prow_ps = pspool.tile([1, CPC], F32, tag="prow_ps")
            nc.tensor.transpose(prow_ps[:, :], pool_own[:, :], shifts_t[0:CPC, 0:CPC])
            nc.scalar.activation(
                smallrow[0:1, POOLCOL:POOLCOL + 16], prow_ps[:, :],
                mybir.ActivationFunctionType.Copy,
            )

            # f = pooled @ W2s.T ; filt = tanh(f + b2)
            f_ps = pspool.tile([1, 36], F32, tag="f_ps")
            nc.tensor.matmul(f_ps[:, :], pooled_col[:, :], w2t_t[:, :])
            fb = spool.tile([1, 36], F32, tag="fb")
            nc.vector.tensor_add(fb[:, :], f_ps[:, :], ptile[0:1, 0:36])
            nc.scalar.activation(
                smallrow[0:1, 0:36], fb[:, :], mybir.ActivationFunctionType.Tanh
            )

            # broadcast table: every smallrow value replicated down 128 partitions
            bct_ps = pspool.tile([128, SRW], F32, tag="bct_ps")
            nc.tensor.matmul(bct_ps[:, :], onesrow[:, :], smallrow[:, :])
            bct = spool.tile([128, SRW], F32, tag="bct")
            nc.scalar.activation(bct[:, :], bct_ps[:, :], mybir.ActivationFunctionType.Copy)
            bcol = spool.tile([128, CPC], F32, tag="bcol")
            nc.vector.tensor_mul(
                bcol[:, :], bct[:, BCCOL:BCCOL + 16], bct[:, POOLCOL:POOLCOL + 16]
            )

            # tridiagonal lhsT builds: per (group, dx) raw filt taps
            traw = {}
            for g in range(4):
                for dxi, dx in enumerate((-1, 0, 1)):
                    wm = FCOL + 9 * g + (dx + 1)
                    w0c = wm + 3
                    wp = wm + 6
                    t1 = tmppool.tile([88, 86], F32, tag="t1")
                    nc.vector.tensor_scalar_mul(
                        t1[:, :], shifts_t[0:88, 0:86], bct[0:88, wm:wm + 1]
                    )
                    t2 = tmppool.tile([88, 86], F32, tag="t2")
                    nc.vector.scalar_tensor_tensor(
                        t2[:, :], shifts_t[0:88, 128:214], bct[0:88, w0c:w0c + 1],
                        t1[:, :], op0=mybir.AluOpType.mult, op1=mybir.AluOpType.add,
                    )
                    tr = tripool.tile([88, 86], F32, tag=f"traw{g}_{dxi}")
                    nc.vector.scalar_tensor_tensor(
                        tr[:, :], shifts_t[0:88, 256:342], bct[0:88, wp:wp + 1],
                        t2[:, :], op0=mybir.AluOpType.mult, op1=mybir.AluOpType.add,
                    )
                    traw[(g, dxi)] = tr
            # per-channel center matrices (residual sigma on the dy=0 diagonal)
            tc0 = {}
            for ch in range(CPC):
                g = ch // 4
                t = tripool.tile([88, 86], F32, tag=f"tc0_{ch}")
                nc.vector.scalar_tensor_tensor(
                    t[:, :], shifts_t[0:88, 128:214], bct[0:88, SIGCOL + ch:SIGCOL + ch + 1],
                    traw[(g, 1)][:, :], op0=mybir.AluOpType.mult, op1=mybir.AluOpType.add,
                )
                tc0[ch] = t

            # main conv per channel
            for ch in range(CPC):
                g = ch // 4
                psa = psa_pool.tile([86, 512], F32, tag="psa")
                psb = psb_pool.tile([85, 256], F32, tag="psb")
                w3 = wch[ch][0:88, :].rearrange("p (w c) -> p w c", c=WBLK)
                for dxi, dx in enumerate((-1, 0, 1)):
                    lt = tc0[ch] if dx == 0 else traw[(g, dxi)]
                    nc.tensor.matmul(
                        psa[:, :],
                        lt[0:88, 0:86],
                        w3[0:88, 0:2, dx + 1:dx + 257],
                        start=(dxi == 0),
                        stop=(dxi == 2),
                    )
                    nc.tensor.matmul(
                        psb[:, :],
                        lt[0:87, 0:85],
                        wch[ch][0:87, 2 * WBLK + dx + 1:2 * WBLK + dx + 257],
                        start=(dxi == 0),
                        stop=(dxi == 2),
                    )
                oa = opool.tile([86, 512], F32, tag="oa")
                ob = opool.tile([85, 256], F32, tag="ob")
                nc.scalar.activation(
                    oa[:, :], psa[:, :], mybir.ActivationFunctionType.Identity,
                    bias=bcol[0:86, ch:ch + 1], scale=bct[0:86, ACOL + ch:ACOL + ch + 1],
                )
                nc.scalar.activation(
                    ob[:, :], psb[:, :], mybir.ActivationFunctionType.Identity,
                    bias=bcol[0:85, ch:ch + 1], scale=bct[0:85, ACOL + ch:ACOL + ch + 1],
                )
                nc.sync.dma_start(out_d[ch, 0:86, :], oa[0:86, 0:256])
                nc.sync.dma_start(out_d[ch, 86:171, :], oa[0:85, 256:512])
                nc.sync.dma_start(out_d[ch, 171:256, :], ob[0:85, 0:256])

    nc.compile()
    return nc


_NC_CACHE = None


def _get_nc():
    global _NC_CACHE
    if _NC_CACHE is None:
        _NC_CACHE = _build_nc()
    return _NC_CACHE


def _shifts_np():
    s = np.zeros((128, SHIFTS_W), np.float32)
    for d in range(3):
        for p in range(128):
            c = p + d
            if c < 128:
                s[c, 128 * d + p] = 1.0
    for wi, (_, _, _, _, _, pv0, pv1, _, _, _) in enumerate(WINDOWS):
        s[pv0:pv1, SELBASE + 15 + 16 * wi] = 1.0
    return s


def kernel(x, conv_w, bn_gamma, bn_beta, bn_mean, bn_var, lamb_l, lamb_h, inside_all):
    x = np.asarray(x, np.float32)
    conv_w = np.asarray(conv_w, np.float32)
    bn_gamma = np.asarray(bn_gamma, np.float32)
    bn_beta = np.asarray(bn_beta, np.float32)
    bn_mean = np.asarray(bn_mean, np.float32)
    bn_var = np.asarray(bn_var, np.float32)
    lamb_l = np.asarray(lamb_l, np.float32)
    lamb_h = np.asarray(lamb_h, np.float32)
    ia = np.asarray(inside_all, np.float32).reshape(C)

    gv = (bn_gamma / np.sqrt(bn_var + np.float32(EPS))).astype(np.float32)
    w2s = (conv_w * gv[:, None] / np.float32(H * W)).astype(np.float32)  # [72, 32]
    b2 = (bn_beta - bn_mean * gv).astype(np.float32)                      # [72]

    A = (lamb_l * (ia + 1.0)).astype(np.float32)
    s = (lamb_h + 1.0).astype(np.float32)
    # device bias multiplies Bc by the pooled SUM, so fold the mean's 1/HW here
    Bc = (-lamb_l * ia / np.float32(H * W)).astype(np.float32)
    A_eff = np.where(A >= 0, np.maximum(A, 1e-20), np.minimum(A, -1e-20)).astype(np.float32)
    sig = (s / A_eff).astype(np.float32)

    shifts = _shifts_np()
    nc = _get_nc()

    in_maps = []
    for core in range(NCORES):
        n = core // 2
        half = core % 2
        csl = slice(16 * half, 16 * half + 16)
        gsl = slice(36 * half, 36 * half + 36)
        params = np.concatenate(
            [b2[gsl], A_eff[csl], sig[csl], Bc[csl], s[csl]]
        ).astype(np.float32).reshape(1, 100)
        in_maps.append({
            "xs": np.ascontiguousarray(x[n, csl]),
            "w2t": np.ascontiguousarray(w2s[gsl].T),
            "params": params,
            "shifts": shifts,
        })

    res = bass_utils.run_bass_kernel_spmd(nc, in_maps, core_ids=list(range(NCORES)))

    out = np.empty((N_B, C, H, W), np.float32)
    for core in range(NCORES):
        n = core // 2
        half = core % 2
        out[n, 16 * half:16 * half + 16] = res.results[core]["out"]
    return out


# revision 7
# speedup vs baseline: 1.1721x; 1.1721x over previous
"""Trainium2 Bass kernel for nn_DynamicFilter (dynamic per-image 3x3 grouped filter).

Math (per batch n, channel c, group g = c//4):
    pooled[n,c] = mean_hw x[n,c]
    f = pooled @ W2.T + b2          (conv1x1 + folded BN)
    filt[n,g,k] = tanh(f)           (k = 9 taps, 3x3, reflect pad)
    out = A_c * conv3x3_{filt[g]}(x) + s_c * x + Bc_c * pooled[n,c]
      A = lamb_l*(inside_all+1), s = lamb_h+1, Bc = -lamb_l*inside_all

Sharding: 8 cores = (n in 0..3) x (channel half in 0..1), 16 channels/core.
The pooled mean needs all 32 channels per n -> tiny pair AllGather (16 floats).

Device mapping per core:
  - x rows (H) on SBUF partitions, one window-tile set per channel with
    1-row overlaps and reflection rows/cols materialized at load.
  - 3x3 conv = 3 matmuls per channel accumulating in PSUM: each lhsT is a
    tridiagonal [in_row x out_row] matrix carrying the 3 vertical taps for
    one horizontal shift dx; rhs free-dim offset provides dx.
  - residual s*x folded into the center tridiagonal diagonal as sigma=s/A;
    per-channel scale A and bias Bc*pooled applied by the ScalarE copy that
    evacuates PSUM.
  - pooling: per-window ones-vector matmuls -> column sums in PSUM ->
    free-dim reduce -> [16,1] partial -> pair AllGather -> [32,1].
"""

import numpy as np

import concourse.bass as bass
import concourse.mybir as mybir
import concourse.tile as tile
from concourse import bacc, bass_utils

F32 = mybir.dt.float32

N_B, C, H, W = 4, 32, 256, 256
CPC = 16   # channels per core
NCORES = 8
EPS = 1e-5

# smallrow / broadcast-table column layout
FCOL = 0     # filt (own 4 groups x 9 taps) : 36
ACOL = 36    # A_eff per own channel        : 16
SIGCOL = 52  # sigma = s/A_eff              : 16
BCCOL = 68   # Bc = -lamb_l*inside_all      : 16
SCOL = 84    # s = lamb_h+1 (unused on dev) : 16
POOLCOL = 100  # own pooled sums            : 16
SRW = 116

# window geometry: (main_src_row0, main_nrows, main_dst_part0,
#                   reflect_src_row, reflect_dst_part,
#                   pool_part0, pool_part1, out_row0, out_nrows, in_parts)
WINDOWS = [
    (0, 87, 1, 1, 0, 1, 87, 0, 86, 88),     # rows -1..86 (part0 = reflect row 1)
    (85, 87, 0, None, None, 1, 86, 86, 85, 87),
    (170, 86, 0, 254, 86, 1, 86, 171, 85, 87),  # part86 = reflect row 254
]
WBLK = 264  # column stride of one window block inside a channel tile
# pooling selector block inside the shifts constant: M[p, 384 + 15 + 16w] is
# the 0/1 row-valid mask of window w; slicing 16 cols starting at
# 384 + 15 + 16w - ch puts that mask at relative column ch, zeros elsewhere.
SELBASE = 384
SHIFTS_W = 448


def _build_nc():
    nc = bacc.Bacc()
    xs = nc.declare_dram_parameter("xs", [CPC, H, W], F32, isOutput=False)
    w2t = nc.declare_dram_parameter("w2t", [C, 36], F32, isOutput=False)
    params = nc.declare_dram_parameter("params", [1, 100], F32, isOutput=False)
    shifts = nc.declare_dram_parameter("shifts", [128, SHIFTS_W], F32, isOutput=False)
    out_d = nc.declare_dram_parameter("out", [CPC, H, W], F32, isOutput=True)

    cc_in = nc.dram_tensor("cc_in", [CPC, 1], F32)
    cc_out = nc.dram_tensor("cc_out", [C, 1], F32)

    CQ = 4           # channels per quad
    NQ = CPC // CQ   # quads
    OBLK = 3 * W     # out-tile columns per channel (w0|w1|w2)

    with tile.TileContext(nc) as tc:
        with (
            tc.tile_pool(name="wbig", bufs=1) as wpool,
            tc.tile_pool(name="consts", bufs=1) as cpool,
            tc.tile_pool(name="small", bufs=1) as spool,
            tc.tile_pool(name="tri", bufs=1) as tripool,
            tc.tile_pool(name="tritmp", bufs=2) as tmppool,
            tc.tile_pool(name="outs", bufs=2) as opool,
            tc.tile_pool(name="ps_small", bufs=1, space="PSUM") as pspool,
            tc.tile_pool(name="ps_a", bufs=2, space="PSUM") as psa_pool,
            tc.tile_pool(name="ps_b", bufs=2, space="PSUM") as psb_pool,
        ):
            # constants
            shifts_t = cpool.tile([128, SHIFTS_W], F32, tag="shifts")
            nc.sync.dma_start(shifts_t[:, :], shifts[:, :])
            w2t_t = cpool.tile([C, 36], F32, tag="w2t")
            nc.sync.dma_start(w2t_t[:, :], w2t[:, :])
            ptile = cpool.tile([1, 100], F32, tag="ptile")
            nc.sync.dma_start(ptile[:, :], params[:, :])
            onesrow = cpool.tile([1, 128], F32, tag="onesrow")
            nc.vector.memset(onesrow[:, :], 1.0)
            smallrow = cpool.tile([1, SRW], F32, tag="smallrow")
            nc.sync.dma_start(smallrow[0:1, ACOL:SCOL + 16], ptile[0:1, 36:100])

            # per-quad window tensors; big SWDGE loads spread across channels
            wq = []
            poolp = pspool.tile([CPC, W], F32, tag="poolp")
            pool_mm = []  # deferred (quad, ch, window) pooling matmul args
            for q in range(NQ):
                t = wpool.tile([128, CQ * 3 * WBLK], F32, tag=f"wq{q}")
                wq.append(t)
                t4 = t[:, :].rearrange("p (c w x) -> p c w x", w=3, x=WBLK)
                xsq = xs[q * CQ:(q + 1) * CQ, :, :]
                for wi, (r0, nr, p0, rr, rp, _, _, _, _, nparts) in enumerate(WINDOWS):
                    nc.gpsimd.dma_start(
                        t4[p0:p0 + nr, 0:CQ, wi, 1:257],
                        xsq[:, r0:r0 + nr, :].rearrange("c r x -> r c x"),
                    )
                    if rr is not None:
                        nc.gpsimd.dma_start(
                            t4[rp:rp + 1, 0:CQ, wi, 1:257],
                            xsq[:, rr:rr + 1, :].rearrange("c r x -> r c x"),
                        )
                # reflect columns: col0 <- col2, col257 <- col255
                nc.vector.tensor_copy(t4[0:88, 0:CQ, 0:3, 0:1], t4[0:88, 0:CQ, 0:3, 2:3])
                nc.vector.tensor_copy(
                    t4[0:88, 0:CQ, 0:3, 257:258], t4[0:88, 0:CQ, 0:3, 255:256]
                )
                # pooling matmuls for this quad (masked selector columns)
                for ci in range(CQ):
                    ch = q * CQ + ci
                    for wi, (_, _, _, _, _, _, _, _, _, nparts) in enumerate(WINDOWS):
                        sel0 = SELBASE + 15 + 16 * wi - ch
                        nc.tensor.matmul(
                            poolp[0:CPC, :],
                            shifts_t[0:nparts, sel0:sel0 + 16],
                            t4[0:nparts, ci, wi, 1:257],
                            start=(ch == 0 and wi == 0),
                            stop=(ch == CPC - 1 and wi == 2),
                        )

            # finish pooling: CS -> reduce -> AllGather
            cs = spool.tile([CPC, W], F32, tag="cs")
            nc.scalar.activation(cs[:, :], poolp[:, :], mybir.ActivationFunctionType.Copy)
            pool_own = spool.tile([CPC, 1], F32, tag="pool_own")
            nc.vector.tensor_reduce(
                pool_own[:, :], cs[:, :], axis=mybir.AxisListType.X, op=mybir.AluOpType.add
            )
            nc.sync.dma_start(cc_in[:, :], pool_own[:, :])
            nc.gpsimd.collective_compute(
                "AllGather",
                mybir.AluOpType.bypass,
                replica_groups=[[0, 1], [2, 3], [4, 5], [6, 7]],
                ins=[cc_in.ap().opt()],
                outs=[cc_out.ap().opt()],
            )
            pooled_col = spool.tile([C, 1], F32, tag="pooled_col")
            nc.sync.dma_start(pooled_col[:, :], cc_out[:, :])

            # own pooled sums as a row (overlaps the collective)
            prow_ps = pspool.tile([1, CPC], F32, tag="prow_ps")
            nc.tensor.transpose(prow_ps[:, :], pool_own[:, :], shifts_t[0:CPC, 0:CPC])
            nc.scalar.activation(
                smallrow[0:1, POOLCOL:POOLCOL + 16], prow_ps[:, :],
                mybir.ActivationFunctionType.Copy,
            )

            # f = pooled @ W2s.T ; filt = tanh(f + b2)
            f_ps = pspool.tile([1, 36], F32, tag="f_ps")
            nc.tensor.matmul(f_ps[:, :], pooled_col[:, :], w2t_t[:, :])
            fb = spool.tile([1, 36], F32, tag="fb")
            nc.vector.tensor_add(fb[:, :], f_ps[:, :], ptile[0:1, 0:36])
            nc.scalar.activation(
                smallrow[0:1, 0:36], fb[:, :], mybir.ActivationFunctionType.Tanh
            )

            # broadcast table: every smallrow value replicated down 128 partitions
            bct_ps = pspool.tile([128, SRW], F32, tag="bct_ps")
            nc.tensor.matmul(bct_ps[:, :], onesrow[:, :], smallrow[:, :])
            bct = spool.tile([128, SRW], F32, tag="bct")
            nc.scalar.activation(bct[:, :], bct_ps[:, :], mybir.ActivationFunctionType.Copy)
            bcol = spool.tile([128, CPC], F32, tag="bcol")
            nc.vector.tensor_mul(
                bcol[:, :], bct[:, BCCOL:BCCOL + 16], bct[:, POOLCOL:POOLCOL + 16]
            )

            # tridiagonal lhsT builds (I_d = identity block shifted d columns)
            traw = {}
            for g in range(4):
                for dxi, dx in enumerate((-1, 0, 1)):
                    wm = FCOL + 9 * g + (dx + 1)
                    w0c = wm + 3
                    wp = wm + 6
                    t1 = tmppool.tile([88, 86], F32, tag="t1")
                    nc.vector.tensor_scalar_mul(
                        t1[:, :], shifts_t[0:88, 0:86], bct[0:88, wm:wm + 1]
                    )
                    t2 = tmppool.tile([88, 86], F32, tag="t2")
                    nc.vector.scalar_tensor_tensor(
                        t2[:, :], shifts_t[0:88, 1:87], bct[0:88, w0c:w0c + 1],
                        t1[:, :], op0=mybir.AluOpType.mult, op1=mybir.AluOpType.add,
                    )
                    tr = tripool.tile([88, 86], F32, tag=f"traw{g}_{dxi}")
                    nc.vector.scalar_tensor_tensor(
                        tr[:, :], shifts_t[0:88, 2:88], bct[0:88, wp:wp + 1],
                        t2[:, :], op0=mybir.AluOpType.mult, op1=mybir.AluOpType.add,
                    )
                    traw[(g, dxi)] = tr
            tc0 = {}
            for ch in range(CPC):
                g = ch // 4
                t = tripool.tile([88, 86], F32, tag=f"tc0_{ch}")
                nc.vector.scalar_tensor_tensor(
                    t[:, :], shifts_t[0:88, 1:87], bct[0:88, SIGCOL + ch:SIGCOL + ch + 1],
                    traw[(g, 1)][:, :], op0=mybir.AluOpType.mult, op1=mybir.AluOpType.add,
                )
                tc0[ch] = t

            # main conv + evac per channel; merged out DMAs per quad
            for q in range(NQ):
                ot = opool.tile([86, CQ * OBLK], F32, tag=f"ot{q % 2}")
                ot4 = ot[:, :].rearrange("p (c w x) -> p c w x", w=3, x=W)
                t4 = wq[q][:, :].rearrange("p (c w x) -> p c w x", w=3, x=WBLK)
                for ci in range(CQ):
                    ch = q * CQ + ci
                    g = ch // 4
                    psa = psa_pool.tile([86, 512], F32, tag="psa")
                    psb = psb_pool.tile([85, 256], F32, tag="psb")
                    for dxi, dx in enumerate((-1, 0, 1)):
                        lt = tc0[ch] if dx == 0 else traw[(g, dxi)]
                        nc.tensor.matmul(
                            psa[:, :],
                            lt[0:88, 0:86],
                            t4[0:88, ci, 0:2, dx + 1:dx + 257],
                            start=(dxi == 0),
                            stop=(dxi == 2),
                        )
                        nc.tensor.matmul(
                            psb[:, :],
                            lt[0:87, 0:85],
                            t4[0:87, ci, 2, dx + 1:dx + 257],
                            start=(dxi == 0),
                            stop=(dxi == 2),
                        )
                    nc.scalar.activation(
                        ot4[0:86, ci, 0:2, :].rearrange("p a b -> p (a b)"),
                        psa[:, :], mybir.ActivationFunctionType.Identity,
                        bias=bcol[0:86, ch:ch + 1],
                        scale=bct[0:86, ACOL + ch:ACOL + ch + 1],
                    )
                    nc.scalar.activation(
                        ot4[0:85, ci, 2, :], psb[:, :],
                        mybir.ActivationFunctionType.Identity,
                        bias=bcol[0:85, ch:ch + 1],
                        scale=bct[0:85, ACOL + ch:ACOL + ch + 1],
                    )
                # out DMAs for the quad
                od = out_d[q * CQ:(q + 1) * CQ, :, :]
                # w0 rows 0..84 and w1 rows 86..170 (DMA APs max 3 dims)
                nc.gpsimd.dma_start(
                    od[:, 0:85, :].rearrange("c r x -> r c x"),
                    ot4[0:85, 0:CQ, 0, :],
                )
                nc.gpsimd.dma_start(
                    od[:, 86:171, :].rearrange("c r x -> r c x"),
                    ot4[0:85, 0:CQ, 1, :],
                )
                # row 85 of w0 block
                nc.gpsimd.dma_start(
                    od[:, 85:86, :].rearrange("c r x -> r c x"),
                    ot4[85:86, 0:CQ, 0, :],
                )
                # w2 block rows 171..255
                nc.gpsimd.dma_start(
                    od[:, 171:256, :].rearrange("c r x -> r c x"),
                    ot4[0:85, 0:CQ, 2, :],
                )

    nc.compile()
    return nc


_NC_CACHE = None


def _get_nc():
    global _NC_CACHE
    if _NC_CACHE is None:
        _NC_CACHE = _build_nc()
    return _NC_CACHE


def _shifts_np():
    s = np.zeros((128, SHIFTS_W), np.float32)
    for d in range(3):
        for p in range(128):
            c = p + d
            if c < 128:
                s[c, 128 * d + p] = 1.0
    for wi, (_, _, _, _, _, pv0, pv1, _, _, _) in enumerate(WINDOWS):
        s[pv0:pv1, SELBASE + 15 + 16 * wi] = 1.0
    return s


def kernel(x, conv_w, bn_gamma, bn_beta, bn_mean, bn_var, lamb_l, lamb_h, inside_all):
    x = np.asarray(x, np.float32)
    conv_w = np.asarray(conv_w, np.float32)
    bn_gamma = np.asarray(bn_gamma, np.float32)
    bn_beta = np.asarray(bn_beta, np.float32)
    bn_mean = np.asarray(bn_mean, np.float32)
    bn_var = np.asarray(bn_var, np.float32)
    lamb_l = np.asarray(lamb_l, np.float32)
    lamb_h = np.asarray(lamb_h, np.float32)
    ia = np.asarray(inside_all, np.float32).reshape(C)

    gv = (bn_gamma / np.sqrt(bn_var + np.float32(EPS))).astype(np.float32)
    w2s = (conv_w * gv[:, None] / np.float32(H * W)).astype(np.float32)  # [72, 32]
    b2 = (bn_beta - bn_mean * gv).astype(np.float32)                      # [72]

    A = (lamb_l * (ia + 1.0)).astype(np.float32)
    s = (lamb_h + 1.0).astype(np.float32)
    # device bias multiplies Bc by the pooled SUM, so fold the mean's 1/HW here
    Bc = (-lamb_l * ia / np.float32(H * W)).astype(np.float32)
    A_eff = np.where(A >= 0, np.maximum(A, 1e-20), np.minimum(A, -1e-20)).astype(np.float32)
    sig = (s / A_eff).astype(np.float32)

    shifts = _shifts_np()
    nc = _get_nc()

    in_maps = []
    for core in range(NCORES):
        n = core // 2
        half = core % 2
        csl = slice(16 * half, 16 * half + 16)
        gsl = slice(36 * half, 36 * half + 36)
        params = np.concatenate(
            [b2[gsl], A_eff[csl], sig[csl], Bc[csl], s[csl]]
        ).astype(np.float32).reshape(1, 100)
        in_maps.append({
            "xs": np.ascontiguousarray(x[n, csl]),
            "w2t": np.ascontiguousarray(w2s[gsl].T),
            "params": params,
            "shifts": shifts,
        })

    res = bass_utils.run_bass_kernel_spmd(nc, in_maps, core_ids=list(range(NCORES)))

    out = np.empty((N_B, C, H, W), np.float32)
    for core in range(NCORES):
        n = core // 2
        half = core % 2
        out[n, 16 * half:16 * half + 16] = res.results[core]["out"]
    return out


# revision 10
# speedup vs baseline: 1.9126x; 1.6318x over previous
"""Trainium2 Bass kernel for nn_DynamicFilter (dynamic per-image 3x3 grouped filter).

Math (per batch n, channel c, group g = c//4):
    pooled[n,c] = mean_hw x[n,c]
    f = pooled @ W2.T + b2          (conv1x1 + folded BN)
    filt[n,g,k] = tanh(f)           (k = 9 taps, 3x3, reflect pad)
    out = A_c * conv3x3_{filt[g]}(x) + s_c * x + Bc_c * pooled[n,c]
      A = lamb_l*(inside_all+1), s = lamb_h+1, Bc = -lamb_l*inside_all

Sharding: 8 cores = (n in 0..3) x (channel half in 0..1), 16 channels/core.
The pooled mean needs all 32 channels per n -> tiny pair AllGather (16 floats).

Device mapping per core:
  - x rows (H) on SBUF partitions, one window-tile set per channel with
    1-row overlaps and reflection rows/cols materialized at load.
  - 3x3 conv = 3 matmuls per channel accumulating in PSUM: each lhsT is a
    tridiagonal [in_row x out_row] matrix carrying the 3 vertical taps for
    one horizontal shift dx; rhs free-dim offset provides dx.
  - residual s*x folded into the center tridiagonal diagonal as sigma=s/A;
    per-channel scale A and bias Bc*pooled applied by the ScalarE copy that
    evacuates PSUM.
  - pooling: per-window ones-vector matmuls -> column sums in PSUM ->
    free-dim reduce -> [16,1] partial -> pair AllGather -> [32,1].
"""

import numpy as np

import concourse.bass as bass
import concourse.mybir as mybir
import concourse.tile as tile
from concourse import bacc, bass_utils

F32 = mybir.dt.float32
F32R = mybir.dt.float32r

N_B, C, H, W = 4, 32, 256, 256
CPC = 16   # channels per core
NCORES = 8
EPS = 1e-5

# smallrow / broadcast-table column layout
FCOL = 0     # filt (own 4 groups x 9 taps) : 36
ACOL = 36    # A_eff per own channel        : 16
SIGCOL = 52  # sigma = s/A_eff              : 16
BCCOL = 68   # Bc = -lamb_l*inside_all      : 16
SCOL = 84    # s = lamb_h+1 (unused on dev) : 16
POOLCOL = 100  # own pooled sums            : 16
SRW = 116

# window geometry: (main_src_row0, main_nrows, main_dst_part0,
#                   reflect_src_row, reflect_dst_part,
#                   pool_part0, pool_part1, out_row0, out_nrows, in_parts)
WINDOWS = [
    (0, 87, 1, 1, 0, 1, 87, 0, 86, 88),     # rows -1..86 (part0 = reflect row 1)
    (85, 87, 0, None, None, 1, 86, 86, 85, 87),
    (170, 86, 0, 254, 86, 1, 86, 171, 85, 87),  # part86 = reflect row 254
]
WBLK = 264  # column stride of one window block inside a channel tile
# pooling selector block inside the shifts constant: M[p, 384 + 15 + 16w] is
# the 0/1 row-valid mask of window w; slicing 16 cols starting at
# 384 + 15 + 16w - ch puts that mask at relative column ch, zeros elsewhere.
SELBASE = 384
SHIFTS_W = 448


def _build_nc():
    nc = bacc.Bacc()
    xs = nc.declare_dram_parameter("xs", [CPC, H, W], F32R, isOutput=False)
    w2t = nc.declare_dram_parameter("w2t", [C, 36], F32, isOutput=False)
    params = nc.declare_dram_parameter("params", [1, 100], F32, isOutput=False)
    shifts = nc.declare_dram_parameter("shifts", [128, SHIFTS_W], F32, isOutput=False)
    selmask = nc.declare_dram_parameter("selmask", [128, 64], F32R, isOutput=False)
    out_d = nc.declare_dram_parameter("out", [CPC, H, W], F32, isOutput=True)

    cc_in = nc.dram_tensor("cc_in", [CPC, 1], F32)
    cc_out = nc.dram_tensor("cc_out", [C, 1], F32)

    CQ = 4           # channels per quad
    NQ = CPC // CQ   # quads
    OBLK = 3 * W     # out-tile columns per channel (w0|w1|w2)

    with tile.TileContext(nc) as tc:
        with (
            tc.tile_pool(name="wbig", bufs=1) as wpool,
            tc.tile_pool(name="consts", bufs=1) as cpool,
            tc.tile_pool(name="small", bufs=1) as spool,
            tc.tile_pool(name="tri", bufs=1) as tripool,
            tc.tile_pool(name="tritmp", bufs=2) as tmppool,
            tc.tile_pool(name="outs", bufs=2) as opool,
            tc.tile_pool(name="ps_small", bufs=1, space="PSUM") as pspool,
            tc.tile_pool(name="ps_a", bufs=2, space="PSUM") as psa_pool,
            tc.tile_pool(name="ps_b", bufs=2, space="PSUM") as psb_pool,
        ):
            # constants
            shifts_t = cpool.tile([128, SHIFTS_W], F32, tag="shifts")
            nc.sync.dma_start(shifts_t[:, :], shifts[:, :])
            w2t_t = cpool.tile([C, 36], F32, tag="w2t")
            nc.sync.dma_start(w2t_t[:, :], w2t[:, :])
            selmask_t = cpool.tile([128, 64], F32R, tag="selmask")
            nc.sync.dma_start(selmask_t[:, :], selmask[:, :])
            ptile = cpool.tile([1, 100], F32, tag="ptile")
            nc.sync.dma_start(ptile[:, :], params[:, :])
            onesrow = cpool.tile([1, 128], F32, tag="onesrow")
            nc.vector.memset(onesrow[:, :], 1.0)
            smallrow = cpool.tile([1, SRW], F32, tag="smallrow")
            nc.sync.dma_start(smallrow[0:1, ACOL:SCOL + 16], ptile[0:1, 36:100])

            # per-quad window tensors; big SWDGE loads spread across channels
            wq = []
            poolp = pspool.tile([CPC, W], F32, tag="poolp")
            pool_mm = []  # deferred (quad, ch, window) pooling matmul args
            for q in range(NQ):
                t = wpool.tile([128, CQ * 3 * WBLK], F32R, tag=f"wq{q}")
                wq.append(t)
                t4 = t[:, :].rearrange("p (c w x) -> p c w x", w=3, x=WBLK)
                xsq = xs[q * CQ:(q + 1) * CQ, :, :]
                for wi, (r0, nr, p0, rr, rp, _, _, _, _, nparts) in enumerate(WINDOWS):
                    nc.gpsimd.dma_start(
                        t4[p0:p0 + nr, 0:CQ, wi, 1:257],
                        xsq[:, r0:r0 + nr, :].rearrange("c r x -> r c x"),
                    )
                    if rr is not None:
                        nc.gpsimd.dma_start(
                            t4[rp:rp + 1, 0:CQ, wi, 1:257],
                            xsq[:, rr:rr + 1, :].rearrange("c r x -> r c x"),
                        )
                # reflect columns: col0 <- col2, col257 <- col255
                nc.vector.tensor_copy(t4[0:88, 0:CQ, 0:3, 0:1], t4[0:88, 0:CQ, 0:3, 2:3])
                nc.vector.tensor_copy(
                    t4[0:88, 0:CQ, 0:3, 257:258], t4[0:88, 0:CQ, 0:3, 255:256]
                )
                # pooling matmuls for this quad (masked selector columns)
                for ci in range(CQ):
                    ch = q * CQ + ci
                    for wi, (_, _, _, _, _, _, _, _, _, nparts) in enumerate(WINDOWS):
                        sel0 = 15 + 16 * wi - ch
                        nc.tensor.matmul(
                            poolp[0:CPC, :],
                            selmask_t[0:nparts, sel0:sel0 + 16],
                            t4[0:nparts, ci, wi, 1:257],
                            start=(ch == 0 and wi == 0),
                            stop=(ch == CPC - 1 and wi == 2),
                        )

            # finish pooling: CS -> reduce -> AllGather
            cs = spool.tile([CPC, W], F32, tag="cs")
            nc.scalar.activation(cs[:, :], poolp[:, :], mybir.ActivationFunctionType.Copy)
            pool_own = spool.tile([CPC, 1], F32, tag="pool_own")
            nc.vector.tensor_reduce(
                pool_own[:, :], cs[:, :], axis=mybir.AxisListType.X, op=mybir.AluOpType.add
            )
            nc.sync.dma_start(cc_in[:, :], pool_own[:, :])
            nc.gpsimd.collective_compute(
                "AllGather",
                mybir.AluOpType.bypass,
                replica_groups=[[0, 1], [2, 3], [4, 5], [6, 7]],
                ins=[cc_in.ap().opt()],
                outs=[cc_out.ap().opt()],
            )
            pooled_col = spool.tile([C, 1], F32, tag="pooled_col")
            nc.sync.dma_start(pooled_col[:, :], cc_out[:, :])

            # own pooled sums as a row (overlaps the collective)
            prow_ps = pspool.tile([1, CPC], F32, tag="prow_ps")
            nc.tensor.transpose(prow_ps[:, :], pool_own[:, :], shifts_t[0:CPC, 0:CPC])
            nc.scalar.activation(
                smallrow[0:1, POOLCOL:POOLCOL + 16], prow_ps[:, :],
                mybir.ActivationFunctionType.Copy,
            )

            # f = pooled @ W2s.T ; filt = tanh(f + b2)
            f_ps = pspool.tile([1, 36], F32, tag="f_ps")
            nc.tensor.matmul(f_ps[:, :], pooled_col[:, :], w2t_t[:, :])
            fb = spool.tile([1, 36], F32, tag="fb")
            nc.vector.tensor_add(fb[:, :], f_ps[:, :], ptile[0:1, 0:36])
            nc.scalar.activation(
                smallrow[0:1, 0:36], fb[:, :], mybir.ActivationFunctionType.Tanh
            )

            # broadcast table: every smallrow value replicated down 128 partitions
            bct_ps = pspool.tile([128, SRW], F32, tag="bct_ps")
            nc.tensor.matmul(bct_ps[:, :], onesrow[:, :], smallrow[:, :])
            bct = spool.tile([128, SRW], F32, tag="bct")
            nc.scalar.activation(bct[:, :], bct_ps[:, :], mybir.ActivationFunctionType.Copy)
            bcol = spool.tile([128, CPC], F32, tag="bcol")
            nc.vector.tensor_mul(
                bcol[:, :], bct[:, BCCOL:BCCOL + 16], bct[:, POOLCOL:POOLCOL + 16]
            )

            # tridiagonal lhsT builds (I_d = identity block shifted d columns)
            traw = {}
            for g in range(4):
                for dxi, dx in enumerate((-1, 0, 1)):
                    wm = FCOL + 9 * g + (dx + 1)
                    w0c = wm + 3
                    wp = wm + 6
                    t1 = tmppool.tile([88, 86], F32, tag="t1")
                    nc.vector.tensor_scalar_mul(
                        t1[:, :], shifts_t[0:88, 0:86], bct[0:88, wm:wm + 1]
                    )
                    t2 = tmppool.tile([88, 86], F32, tag="t2")
                    nc.vector.scalar_tensor_tensor(
                        t2[:, :], shifts_t[0:88, 1:87], bct[0:88, w0c:w0c + 1],
                        t1[:, :], op0=mybir.AluOpType.mult, op1=mybir.AluOpType.add,
                    )
                    tr = tripool.tile([88, 86], F32R, tag=f"traw{g}_{dxi}")
                    nc.vector.scalar_tensor_tensor(
                        tr[:, :], shifts_t[0:88, 2:88], bct[0:88, wp:wp + 1],
                        t2[:, :], op0=mybir.AluOpType.mult, op1=mybir.AluOpType.add,
                    )
                    traw[(g, dxi)] = tr
            tc0 = {}
            for ch in range(CPC):
                g = ch // 4
                t = tripool.tile([88, 86], F32R, tag=f"tc0_{ch}")
                nc.vector.scalar_tensor_tensor(
                    t[:, :], shifts_t[0:88, 1:87], bct[0:88, SIGCOL + ch:SIGCOL + ch + 1],
                    traw[(g, 1)][:, :], op0=mybir.AluOpType.mult, op1=mybir.AluOpType.add,
                )
                tc0[ch] = t

            # main conv + evac per channel; merged out DMAs per quad
            for q in range(NQ):
                ot = opool.tile([86, CQ * OBLK], F32, tag=f"ot{q % 2}")
                ot4 = ot[:, :].rearrange("p (c w x) -> p c w x", w=3, x=W)
                t4 = wq[q][:, :].rearrange("p (c w x) -> p c w x", w=3, x=WBLK)
                for ci in range(CQ):
                    ch = q * CQ + ci
                    g = ch // 4
                    psa = psa_pool.tile([86, 512], F32, tag="psa")
                    psb = psb_pool.tile([85, 256], F32, tag="psb")
                    for dxi, dx in enumerate((-1, 0, 1)):
                        lt = tc0[ch] if dx == 0 else traw[(g, dxi)]
                        nc.tensor.matmul(
                            psa[:, :],
                            lt[0:88, 0:86],
                            t4[0:88, ci, 0:2, dx + 1:dx + 257],
                            start=(dxi == 0),
                            stop=(dxi == 2),
                        )
                        nc.tensor.matmul(
                            psb[:, :],
                            lt[0:87, 0:85],
                            t4[0:87, ci, 2, dx + 1:dx + 257],
                            start=(dxi == 0),
                            stop=(dxi == 2),
                        )
                    nc.scalar.activation(
                        ot4[0:86, ci, 0:2, :].rearrange("p a b -> p (a b)"),
                        psa[:, :], mybir.ActivationFunctionType.Identity,
                        bias=bcol[0:86, ch:ch + 1],
                        scale=bct[0:86, ACOL + ch:ACOL + ch + 1],
                    )
                    nc.scalar.activation(
                        ot4[0:85, ci, 2, :], psb[:, :],
                        mybir.ActivationFunctionType.Identity,
                        bias=bcol[0:85, ch:ch + 1],
                        scale=bct[0:85, ACOL + ch:ACOL + ch + 1],
                    )
                # out DMAs for the quad
                od = out_d[q * CQ:(q + 1) * CQ, :, :]
                # w0 rows 0..84 and w1 rows 86..170 (DMA APs max 3 dims)
                nc.gpsimd.dma_start(
                    od[:, 0:85, :].rearrange("c r x -> r c x"),
                    ot4[0:85, 0:CQ, 0, :],
                )
                nc.gpsimd.dma_start(
                    od[:, 86:171, :].rearrange("c r x -> r c x"),
                    ot4[0:85, 0:CQ, 1, :],
                )
                # row 85 of w0 block
                nc.gpsimd.dma_start(
                    od[:, 85:86, :].rearrange("c r x -> r c x"),
                    ot4[85:86, 0:CQ, 0, :],
                )
                # w2 block rows 171..255
                nc.gpsimd.dma_start(
                    od[:, 171:256, :].rearrange("c r x -> r c x"),
                    ot4[0:85, 0:CQ, 2, :],
                )

    nc.compile()
    return nc


_NC_CACHE = None


def _get_nc():
    global _NC_CACHE
    if _NC_CACHE is None:
        _NC_CACHE = _build_nc()
    return _NC_CACHE


def _selmask_np():
    s = np.zeros((128, 64), np.float32)
    for wi, (_, _, _, _, _, pv0, pv1, _, _, _) in enumerate(WINDOWS):
        s[pv0:pv1, 15 + 16 * wi] = 1.0
    return s


def _shifts_np():
    s = np.zeros((128, SHIFTS_W), np.float32)
    for d in range(3):
        for p in range(128):
            c = p + d
            if c < 128:
                s[c, 128 * d + p] = 1.0
    for wi, (_, _, _, _, _, pv0, pv1, _, _, _) in enumerate(WINDOWS):
        s[pv0:pv1, SELBASE + 15 + 16 * wi] = 1.0
    return s


def kernel(x, conv_w, bn_gamma, bn_beta, bn_mean, bn_var, lamb_l, lamb_h, inside_all):
    x = np.asarray(x, np.float32)
    conv_w = np.asarray(conv_w, np.float32)
    bn_gamma = np.asarray(bn_gamma, np.float32)
    bn_beta = np.asarray(bn_beta, np.float32)
    bn_mean = np.asarray(bn_mean, np.float32)
    bn_var = np.asarray(bn_var, np.float32)
    lamb_l = np.asarray(lamb_l, np.float32)
    lamb_h = np.asarray(lamb_h, np.float32)
    ia = np.asarray(inside_all, np.float32).reshape(C)

    gv = (bn_gamma / np.sqrt(bn_var + np.float32(EPS))).astype(np.float32)
    w2s = (conv_w * gv[:, None] / np.float32(H * W)).astype(np.float32)  # [72, 32]
    b2 = (bn_beta - bn_mean * gv).astype(np.float32)                      # [72]

    A = (lamb_l * (ia + 1.0)).astype(np.float32)
    s = (lamb_h + 1.0).astype(np.float32)
    # device bias multiplies Bc by the pooled SUM, so fold the mean's 1/HW here
    Bc = (-lamb_l * ia / np.float32(H * W)).astype(np.float32)
    A_eff = np.where(A >= 0, np.maximum(A, 1e-20), np.minimum(A, -1e-20)).astype(np.float32)
    sig = (s / A_eff).astype(np.float32)

    shifts = _shifts_np()
    selmask = _selmask_np()
    nc = _get_nc()

    in_maps = []
    for core in range(NCORES):
        n = core // 2
        half = core % 2
        csl = slice(16 * half, 16 * half + 16)
        gsl = slice(36 * half, 36 * half + 36)
        params = np.concatenate(
            [b2[gsl], A_eff[csl], sig[csl], Bc[csl], s[csl]]
        ).astype(np.float32).reshape(1, 100)
        in_maps.append({
            "xs": np.ascontiguousarray(x[n, csl]),
            "w2t": np.ascontiguousarray(w2s[gsl].T),
            "params": params,
            "shifts": shifts,
            "selmask": selmask,
        })

    res = bass_utils.run_bass_kernel_spmd(nc, in_maps, core_ids=list(range(NCORES)))

    out = np.empty((N_B, C, H, W), np.float32)
    for core in range(NCORES):
        n = core // 2
        half = core % 2
        out[n, 16 * half:16 * half + 16] = res.results[core]["out"]
    return out
